# revision 34
# baseline (speedup 1.0000x reference)
"""Balanced CE loss kernel for Trainium2 (8 NeuronCores, data parallel).

Math recap of the reference:
  - ce[b,n] = -log_softmax(inputs[b,n,:2])[target[b,n]]
            = softplus((x0-x1) * (2*t-1))           (two-class CE)
  - scores = uniform(key(42), (B,N))  -- a COMPILE-TIME CONSTANT
  - per row: mean of ce over the top-`num_pos`-by-score positives and the
    top-`num_neg`-by-score negatives; valid-count capped by count_pos.
  - loss = mean_b 0.5 * (pos_mean + neg_mean)

Key reductions:
  1. Only positions among each row's top-K (K=256) constant score order can
     be selected, so only those positions of inputs/target matter.
  2. count_pos only enters via min(count_pos, num_pos) and
     min((count_pos*num_neg)//num_pos, num_neg).  If the K-prefix already
     holds >= num_pos positives and >= num_neg negatives (checked EXACTLY on
     the host from the gathered prefix; bit-exact fallback otherwise), both
     saturate to num_pos / num_neg and the full count is never needed.

So each core only computes, for its 16 rows: ce over the K-prefix, a
hardware prefix-scan selection of the first num_pos positives / num_neg
negatives, and the two masked row sums.  The host does the constant
score-order gather and the final 128-row scalar math.
"""

import numpy as np

B, N, C = 128, 131072, 2
NCORES = 8
ROWS = B // NCORES  # 16 rows per core
K = 256             # score-order prefix depth per row

_cache = {}


def _perm():
    """[B, K] int64: first K positions of each row in score-descending order.

    Must match jax.lax.top_k tie-breaking on the reference's scores exactly,
    so compute it with jax.lax.top_k on the very same scores (CPU backend;
    threefry PRNG is backend-deterministic).
    """
    if "perm" not in _cache:
        import jax

        cpu = jax.devices("cpu")[0]
        with jax.default_device(cpu):
            scores = jax.random.uniform(jax.random.key(42), (B, N), dtype=jax.numpy.float32)
            _, idx = jax.lax.top_k(scores, K)
        _cache["perm"] = np.asarray(jax.device_get(idx)).astype(np.int64)
    return _cache["perm"]


def _build_nc(num_pos: int, num_neg: int):
    """Compile the single-core Bass program (same NEFF on all 8 cores)."""
    key = ("nc", num_pos, num_neg)
    if key in _cache:
        return _cache[key]

    import concourse.bacc as bacc
    import concourse.bass as bass
    import concourse.mybir as mybir
    import concourse.tile as tile

    dt = mybir.dt
    af = mybir.ActivationFunctionType
    alu = mybir.AluOpType

    # Steer the ACT-table pass: by default it picks `exp_and_others` for Exp
    # and `natural_log` for Ln, which evict each other (1.28us reload on the
    # critical path).  Restrict Exp/Ln to the combined
    # `natural_log_exp_and_others` set (keeping every set's index intact so
    # act_func_set_id stays valid) -> a single table load serves both.
    if not _cache.get("act_tables_patched"):
        orig_get = bacc.get_activation_tables

        def _combined_tables(arch):
            tabs = orig_get(arch)
            combined = "natural_log_exp_and_others"
            if combined in tabs and {af.Exp, af.Ln} <= tabs[combined]:
                for name, fns in tabs.items():
                    if name != combined:
                        fns.discard(af.Exp)
                        fns.discard(af.Ln)
            return tabs

        bacc.get_activation_tables = _combined_tables
        _cache["act_tables_patched"] = True

    nc = bacc.Bacc("TRN2", target_bir_lowering=False, debug=False)

    # pk packs [t_as_f32 | x0 | x1 | iota] along the free dim -> one input DMA
    pk = nc.dram_tensor("pk", [ROWS, 4 * K], dt.float32, kind="ExternalInput")
    # gtT: transposed prefix target [K, ROWS] as two 128-partition chunks
    # [128, 2*ROWS]; U: constant upper-triangular ones [128, 2, K]
    # (U[p, c, i] = 1 iff c*128+p <= i) for the PE cumsum.
    gtT = nc.dram_tensor("gtT", [128, (K // 128) * ROWS], dt.float32,
                         kind="ExternalInput")
    tri = nc.dram_tensor("tri", [128, K // 128, K], dt.float32,
                         kind="ExternalInput")
    out = nc.dram_tensor("out", [ROWS, 2], dt.float32, kind="ExternalOutput")

    NCHUNK = K // 128

    with tile.TileContext(nc) as tc:
        with (
            tc.tile_pool(name="small", bufs=1) as sp,
            tc.tile_pool(name="psum", bufs=1, space=bass.MemorySpace.PSUM) as pp,
        ):
            pkt = sp.tile([ROWS, 4 * K], dt.float32)
            nc.sync.dma_start(pkt[:], pk.ap())
            gtTt = sp.tile([128, NCHUNK * ROWS], dt.float32)
            nc.gpsimd.dma_start(gtTt[:], gtT.ap())
            trit = sp.tile([128, NCHUNK, K], dt.float32)
            nc.gpsimd.dma_start(trit[:], tri.ap())
            tf = pkt[:, 0:K]
            x0 = pkt[:, K:2 * K]
            x1 = pkt[:, 2 * K:3 * K]
            iota = pkt[:, 3 * K:4 * K]

            # cpos[b, i] = sum_{j <= i} tf[b, j] on the (idle) PE:
            # out[m=b, n=i] = sum_chunks gtT[j, b].T @ U[j, i]
            cpos = pp.tile([ROWS, K], dt.float32)
            for c in range(NCHUNK):
                nc.tensor.matmul(
                    cpos[:],
                    gtTt[:, c * ROWS:(c + 1) * ROWS],
                    trit[:, c, :],
                    start=(c == 0), stop=(c == NCHUNK - 1),
                )

            # ce chain (DVE feeds ACT as early as possible)
            d = sp.tile([ROWS, K], dt.float32)
            nc.vector.tensor_sub(d[:], x0, x1)
            sgn = sp.tile([ROWS, K], dt.float32)
            nc.vector.tensor_scalar(sgn[:], tf, 2.0, -1.0, op0=alu.mult, op1=alu.add)
            dd = sp.tile([ROWS, K], dt.float32)
            nc.vector.tensor_mul(dd[:], d[:], sgn[:])
            # softplus(dd) = relu(dd) + ln(1 + exp(-|dd|)), |dd| = 2*relu-dd
            rl = sp.tile([ROWS, K], dt.float32)
            nc.vector.tensor_scalar_max(rl[:], dd[:], 0.0)
            ab = sp.tile([ROWS, K], dt.float32)
            nc.vector.scalar_tensor_tensor(
                ab[:], rl[:], 2.0, dd[:], op0=alu.mult, op1=alu.subtract
            )
            ex = sp.tile([ROWS, K], dt.float32)
            nc.scalar.activation(ex[:], ab[:], af.Exp, scale=-1.0)
            ln = sp.tile([ROWS, K], dt.float32)
            nc.scalar.activation(ln[:], ex[:], af.Ln, bias=1.0)

            # selection masks; cneg[b,i] = (i+1) - cpos[b,i]
            tn = sp.tile([ROWS, K], dt.float32)
            nc.vector.tensor_scalar(tn[:], tf, -1.0, 1.0, op0=alu.mult, op1=alu.add)
            cneg = sp.tile([ROWS, K], dt.float32)
            nc.vector.scalar_tensor_tensor(
                cneg[:], cpos[:], -1.0, iota, op0=alu.mult, op1=alu.add
            )
            maskp = sp.tile([ROWS, K], dt.float32)
            nc.vector.scalar_tensor_tensor(
                maskp[:], cpos[:], float(num_pos), tf, op0=alu.is_le, op1=alu.mult
            )
            maskn = sp.tile([ROWS, K], dt.float32)
            nc.vector.scalar_tensor_tensor(
                maskn[:], cneg[:], float(num_neg), tn[:], op0=alu.is_le, op1=alu.mult
            )

            ce = sp.tile([ROWS, K], dt.float32)
            nc.vector.tensor_add(ce[:], rl[:], ln[:])

            outsb = sp.tile([ROWS, 2], dt.float32)
            junk0 = sp.tile([ROWS, K], dt.float32)
            nc.vector.scalar_tensor_tensor(
                junk0[:], ce[:], 1.0, maskp[:],
                op0=alu.mult, op1=alu.mult, accum_out=outsb[:, 0:1],
            )
            junk1 = sp.tile([ROWS, K], dt.float32)
            nc.vector.scalar_tensor_tensor(
                junk1[:], ce[:], 1.0, maskn[:],
                op0=alu.mult, op1=alu.mult, accum_out=outsb[:, 1:2],
            )

            nc.sync.dma_start(out.ap(), outsb[:])

    nc.compile()
    _cache[key] = nc
    return nc


def _host_exact(inputs, target, num_pos, num_neg):
    """Exact replication of the reference (jax on CPU). Safety fallback only."""
    import jax
    import jax.numpy as jnp

    cpu = jax.devices("cpu")[0]
    with jax.default_device(cpu):
        inputs = jnp.asarray(inputs)
        target = jnp.asarray(target)
        scores = jax.random.uniform(jax.random.key(42), (B, N))
        is_pos = target == 1
        is_neg = target == 0
        count_pos = is_pos.sum(axis=-1)
        min_pos = jnp.minimum(count_pos, num_pos)
        min_neg = jnp.minimum((count_pos * num_neg) // num_pos, num_neg)
        logp = jax.nn.log_softmax(inputs, axis=-1)
        ce = -jnp.take_along_axis(logp, target[..., None], axis=-1)[..., 0]

        def sampled_mean(mask, k, min_k):
            s = jnp.where(mask, scores, -jnp.inf)
            _, idx = jax.lax.top_k(s, k)
            sel = jnp.take_along_axis(ce, idx, axis=-1)
            valid = jnp.arange(k)[None, :] < min_k[:, None]
            return jnp.where(valid, sel, 0.0).sum(axis=-1) / jnp.maximum(min_k, 1)

        pos_loss = sampled_mean(is_pos, num_pos, min_pos)
        neg_loss = sampled_mean(is_neg, num_neg, min_neg)
        res = ((pos_loss + neg_loss) * 0.5).mean()
    return np.asarray(jax.device_get(res)).astype(np.float32)


def kernel(**inputs) -> np.ndarray:
    from concourse.bass_utils import run_bass_kernel_spmd

    x = np.ascontiguousarray(np.asarray(inputs["inputs"], dtype=np.float32))
    target = np.ascontiguousarray(np.asarray(inputs["target"], dtype=np.int32))
    num_pos = int(np.asarray(inputs["num_pos"]))
    num_neg = int(np.asarray(inputs["num_neg"]))

    if num_pos <= 0 or num_pos > K or num_neg < 0 or num_neg > K:
        # degenerate configs the device program doesn't cover
        return _host_exact(x, target, num_pos, num_neg)

    perm = _perm()
    gt = np.take_along_axis(target, perm, axis=1)          # [B, K] int32
    # Guard: with >= num_pos positives and >= num_neg negatives inside every
    # row's K-prefix, min_pos == num_pos and min_neg == num_neg exactly
    # ((c*nn)//np >= nn  <=>  c >= np for nn > 0), the selected samples all
    # lie inside the prefix, and count_pos is never needed.  Fall back to
    # the exact host computation otherwise (never fires for this data:
    # binomial(256, 1/2) tails; real-data margins are >= 100 of each).
    prefix_pos = gt.sum(axis=1, dtype=np.int64)
    prefix_neg = K - prefix_pos
    if (prefix_pos < num_pos).any() or (prefix_neg < num_neg).any():
        return _host_exact(x, target, num_pos, num_neg)

    pk = np.empty((B, 4 * K), dtype=np.float32)
    pk[:, 0:K] = gt
    pk[:, K:2 * K] = np.take_along_axis(x[:, :, 0], perm, axis=1)
    pk[:, 2 * K:3 * K] = np.take_along_axis(x[:, :, 1], perm, axis=1)
    pk[:, 3 * K:4 * K] = np.arange(1, K + 1, dtype=np.float32)

    NCHUNK = K // 128
    gtf = pk[:, 0:K]  # f32 view of the gathered target
    # tri[p, c, i] = 1 iff c*128+p <= i  (constant, shared by all cores)
    j = (np.arange(128)[:, None, None] + 128 * np.arange(NCHUNK)[None, :, None])
    tri = (j <= np.arange(K)[None, None, :]).astype(np.float32)
    tri = np.ascontiguousarray(tri)

    nc = _build_nc(num_pos, num_neg)
    core_ids = list(range(NCORES))
    in_maps = []
    for c in core_ids:
        rows = gtf[c * ROWS:(c + 1) * ROWS]                      # [ROWS, K]
        gtT = rows.T.reshape(NCHUNK, 128, ROWS).transpose(1, 0, 2)
        in_maps.append({
            "pk": np.ascontiguousarray(pk[c * ROWS:(c + 1) * ROWS]),
            "gtT": np.ascontiguousarray(gtT.reshape(128, NCHUNK * ROWS)),
            "tri": tri,
        })
    res = run_bass_kernel_spmd(nc, in_maps, core_ids, trace=_cache.get("trace", False))
    _cache["last_res"] = res
    outs = np.concatenate([res.results[c]["out"] for c in core_ids], axis=0)  # [B,2]

    pos_loss = outs[:, 0].astype(np.float32) / np.float32(num_pos)
    neg_loss = outs[:, 1].astype(np.float32) / np.float32(max(num_neg, 1))
    loss = np.float32(0.5) * (pos_loss + neg_loss)
    return np.asarray(loss.mean(), dtype=np.float32)


# revision 35
# speedup vs baseline: 1.0422x; 1.0422x over previous
"""Balanced CE loss kernel for Trainium2 (8 NeuronCores, data parallel).

Math recap of the reference:
  - ce[b,n] = -log_softmax(inputs[b,n,:2])[target[b,n]]
            = softplus((x0-x1) * (2*t-1))           (two-class CE)
  - scores = uniform(key(42), (B,N))  -- a COMPILE-TIME CONSTANT
  - per row: mean of ce over the top-`num_pos`-by-score positives and the
    top-`num_neg`-by-score negatives; valid-count capped by count_pos.
  - loss = mean_b 0.5 * (pos_mean + neg_mean)

Key reductions:
  1. Only positions among each row's top-K (K=256) constant score order can
     be selected, so only those positions of inputs/target matter.
  2. count_pos only enters via min(count_pos, num_pos) and
     min((count_pos*num_neg)//num_pos, num_neg).  If the K-prefix already
     holds >= num_pos positives and >= num_neg negatives (checked EXACTLY on
     the host from the gathered prefix; bit-exact fallback otherwise), both
     saturate to num_pos / num_neg and the full count is never needed.

So each core only computes, for its 16 rows: ce over the K-prefix, a
hardware prefix-scan selection of the first num_pos positives / num_neg
negatives, and the two masked row sums.  The host does the constant
score-order gather and the final 128-row scalar math.
"""

import numpy as np

B, N, C = 128, 131072, 2
NCORES = 8
ROWS = B // NCORES  # 16 rows per core
K = 256             # score-order prefix depth per row

_cache = {}


def _perm():
    """[B, K] int64: first K positions of each row in score-descending order.

    Must match jax.lax.top_k tie-breaking on the reference's scores exactly,
    so compute it with jax.lax.top_k on the very same scores (CPU backend;
    threefry PRNG is backend-deterministic).
    """
    if "perm" not in _cache:
        import jax

        cpu = jax.devices("cpu")[0]
        with jax.default_device(cpu):
            scores = jax.random.uniform(jax.random.key(42), (B, N), dtype=jax.numpy.float32)
            _, idx = jax.lax.top_k(scores, K)
        _cache["perm"] = np.asarray(jax.device_get(idx)).astype(np.int64)
    return _cache["perm"]


def _build_nc(num_pos: int, num_neg: int):
    """Compile the single-core Bass program (same NEFF on all 8 cores)."""
    key = ("nc", num_pos, num_neg)
    if key in _cache:
        return _cache[key]

    import concourse.bacc as bacc
    import concourse.bass as bass
    import concourse.mybir as mybir
    import concourse.tile as tile

    dt = mybir.dt
    af = mybir.ActivationFunctionType
    alu = mybir.AluOpType

    # Steer the ACT-table pass: by default it picks `exp_and_others` for Exp
    # and `natural_log` for Ln, which evict each other (1.28us reload on the
    # critical path).  Restrict Exp/Ln to the combined
    # `natural_log_exp_and_others` set (keeping every set's index intact so
    # act_func_set_id stays valid) -> a single table load serves both.
    if not _cache.get("act_tables_patched"):
        orig_get = bacc.get_activation_tables

        def _combined_tables(arch):
            tabs = orig_get(arch)
            combined = "natural_log_exp_and_others"
            if combined in tabs and {af.Exp, af.Ln} <= tabs[combined]:
                for name, fns in tabs.items():
                    if name != combined:
                        fns.discard(af.Exp)
                        fns.discard(af.Ln)
            return tabs

        bacc.get_activation_tables = _combined_tables
        _cache["act_tables_patched"] = True

    nc = bacc.Bacc("TRN2", target_bir_lowering=False, debug=False)

    # pk packs [t_as_f32 | x0 | x1] along the free dim -> one input DMA
    pk = nc.dram_tensor("pk", [ROWS, 3 * K], dt.float32, kind="ExternalInput")
    out = nc.dram_tensor("out", [ROWS, 2], dt.float32, kind="ExternalOutput")

    with tile.TileContext(nc) as tc:
        with tc.tile_pool(name="small", bufs=1) as sp:
            zeros = sp.tile([ROWS, K], dt.float32)
            nc.gpsimd.memset(zeros[:], 0.0)

            pkt = sp.tile([ROWS, 3 * K], dt.float32)
            nc.sync.dma_start(pkt[:], pk.ap())
            tf = pkt[:, 0:K]
            x0 = pkt[:, K:2 * K]
            x1 = pkt[:, 2 * K:3 * K]

            # ce chain (DVE feeds ACT as early as possible)
            d = sp.tile([ROWS, K], dt.float32)
            nc.vector.tensor_sub(d[:], x0, x1)
            sgn = sp.tile([ROWS, K], dt.float32)
            nc.vector.tensor_scalar(sgn[:], tf, 2.0, -1.0, op0=alu.mult, op1=alu.add)
            dd = sp.tile([ROWS, K], dt.float32)
            nc.vector.tensor_mul(dd[:], d[:], sgn[:])
            # softplus(dd) = relu(dd) + ln(1 + exp(-|dd|)), |dd| = 2*relu-dd
            rl = sp.tile([ROWS, K], dt.float32)
            nc.vector.tensor_scalar_max(rl[:], dd[:], 0.0)
            ab = sp.tile([ROWS, K], dt.float32)
            nc.vector.scalar_tensor_tensor(
                ab[:], rl[:], 2.0, dd[:], op0=alu.mult, op1=alu.subtract
            )
            ex = sp.tile([ROWS, K], dt.float32)
            nc.scalar.activation(ex[:], ab[:], af.Exp, scale=-1.0)
            ln = sp.tile([ROWS, K], dt.float32)
            nc.scalar.activation(ln[:], ex[:], af.Ln, bias=1.0)

            # selection masks (need only tf -- run on DVE while ACT works)
            tn = sp.tile([ROWS, K], dt.float32)
            nc.vector.tensor_scalar(tn[:], tf, -1.0, 1.0, op0=alu.mult, op1=alu.add)
            cpos = sp.tile([ROWS, K], dt.float32)
            nc.vector.tensor_tensor_scan(
                cpos[:], tf, zeros[:], 0.0, op0=alu.add, op1=alu.add
            )
            cneg = sp.tile([ROWS, K], dt.float32)
            nc.vector.tensor_tensor_scan(
                cneg[:], tn[:], zeros[:], 0.0, op0=alu.add, op1=alu.add
            )
            maskp = sp.tile([ROWS, K], dt.float32)
            nc.vector.scalar_tensor_tensor(
                maskp[:], cpos[:], float(num_pos), tf, op0=alu.is_le, op1=alu.mult
            )
            maskn = sp.tile([ROWS, K], dt.float32)
            nc.vector.scalar_tensor_tensor(
                maskn[:], cneg[:], float(num_neg), tn[:], op0=alu.is_le, op1=alu.mult
            )

            ce = sp.tile([ROWS, K], dt.float32)
            nc.vector.tensor_add(ce[:], rl[:], ln[:])

            outsb = sp.tile([ROWS, 2], dt.float32)
            junk0 = sp.tile([ROWS, K], dt.float32)
            nc.vector.scalar_tensor_tensor(
                junk0[:], ce[:], 1.0, maskp[:],
                op0=alu.mult, op1=alu.mult, accum_out=outsb[:, 0:1],
            )
            junk1 = sp.tile([ROWS, K], dt.float32)
            nc.vector.scalar_tensor_tensor(
                junk1[:], ce[:], 1.0, maskn[:],
                op0=alu.mult, op1=alu.mult, accum_out=outsb[:, 1:2],
            )

            nc.sync.dma_start(out.ap(), outsb[:])

    nc.compile()
    _cache[key] = nc
    return nc


def _host_exact(inputs, target, num_pos, num_neg):
    """Exact replication of the reference (jax on CPU). Safety fallback only."""
    import jax
    import jax.numpy as jnp

    cpu = jax.devices("cpu")[0]
    with jax.default_device(cpu):
        inputs = jnp.asarray(inputs)
        target = jnp.asarray(target)
        scores = jax.random.uniform(jax.random.key(42), (B, N))
        is_pos = target == 1
        is_neg = target == 0
        count_pos = is_pos.sum(axis=-1)
        min_pos = jnp.minimum(count_pos, num_pos)
        min_neg = jnp.minimum((count_pos * num_neg) // num_pos, num_neg)
        logp = jax.nn.log_softmax(inputs, axis=-1)
        ce = -jnp.take_along_axis(logp, target[..., None], axis=-1)[..., 0]

        def sampled_mean(mask, k, min_k):
            s = jnp.where(mask, scores, -jnp.inf)
            _, idx = jax.lax.top_k(s, k)
            sel = jnp.take_along_axis(ce, idx, axis=-1)
            valid = jnp.arange(k)[None, :] < min_k[:, None]
            return jnp.where(valid, sel, 0.0).sum(axis=-1) / jnp.maximum(min_k, 1)

        pos_loss = sampled_mean(is_pos, num_pos, min_pos)
        neg_loss = sampled_mean(is_neg, num_neg, min_neg)
        res = ((pos_loss + neg_loss) * 0.5).mean()
    return np.asarray(jax.device_get(res)).astype(np.float32)


def kernel(**inputs) -> np.ndarray:
    from concourse.bass_utils import run_bass_kernel_spmd

    x = np.ascontiguousarray(np.asarray(inputs["inputs"], dtype=np.float32))
    target = np.ascontiguousarray(np.asarray(inputs["target"], dtype=np.int32))
    num_pos = int(np.asarray(inputs["num_pos"]))
    num_neg = int(np.asarray(inputs["num_neg"]))

    if num_pos <= 0 or num_pos > K or num_neg < 0 or num_neg > K:
        # degenerate configs the device program doesn't cover
        return _host_exact(x, target, num_pos, num_neg)

    perm = _perm()
    gt = np.take_along_axis(target, perm, axis=1)          # [B, K] int32
    # Guard: with >= num_pos positives and >= num_neg negatives inside every
    # row's K-prefix, min_pos == num_pos and min_neg == num_neg exactly
    # ((c*nn)//np >= nn  <=>  c >= np for nn > 0), the selected samples all
    # lie inside the prefix, and count_pos is never needed.  Fall back to
    # the exact host computation otherwise (never fires for this data:
    # binomial(256, 1/2) tails; real-data margins are >= 100 of each).
    prefix_pos = gt.sum(axis=1, dtype=np.int64)
    prefix_neg = K - prefix_pos
    if (prefix_pos < num_pos).any() or (prefix_neg < num_neg).any():
        return _host_exact(x, target, num_pos, num_neg)

    pk = np.empty((B, 3 * K), dtype=np.float32)
    pk[:, 0:K] = gt
    pk[:, K:2 * K] = np.take_along_axis(x[:, :, 0], perm, axis=1)
    pk[:, 2 * K:3 * K] = np.take_along_axis(x[:, :, 1], perm, axis=1)

    nc = _build_nc(num_pos, num_neg)
    core_ids = list(range(NCORES))
    in_maps = [
        {"pk": np.ascontiguousarray(pk[c * ROWS:(c + 1) * ROWS])}
        for c in core_ids
    ]
    res = run_bass_kernel_spmd(nc, in_maps, core_ids, trace=_cache.get("trace", False))
    _cache["last_res"] = res
    outs = np.concatenate([res.results[c]["out"] for c in core_ids], axis=0)  # [B,2]

    pos_loss = outs[:, 0].astype(np.float32) / np.float32(num_pos)
    neg_loss = outs[:, 1].astype(np.float32) / np.float32(max(num_neg, 1))
    loss = np.float32(0.5) * (pos_loss + neg_loss)
    return np.asarray(loss.mean(), dtype=np.float32)


# revision 36
# speedup vs baseline: 1.0804x; 1.0367x over previous
"""Balanced CE loss kernel for Trainium2 (8 NeuronCores, data parallel).

Math recap of the reference:
  - ce[b,n] = -log_softmax(inputs[b,n,:2])[target[b,n]]
            = softplus((x0-x1) * (2*t-1))           (two-class CE)
  - scores = uniform(key(42), (B,N))  -- a COMPILE-TIME CONSTANT
  - per row: mean of ce over the top-`num_pos`-by-score positives and the
    top-`num_neg`-by-score negatives; valid-count capped by count_pos.
  - loss = mean_b 0.5 * (pos_mean + neg_mean)

Key reductions:
  1. Only positions among each row's top-K (K=256) constant score order can
     be selected, so only those positions of inputs/target matter.
  2. count_pos only enters via min(count_pos, num_pos) and
     min((count_pos*num_neg)//num_pos, num_neg).  If the K-prefix already
     holds >= num_pos positives and >= num_neg negatives (checked EXACTLY on
     the host from the gathered prefix; bit-exact fallback otherwise), both
     saturate to num_pos / num_neg and the full count is never needed.

So each core only computes, for its 16 rows: ce over the K-prefix, a
hardware prefix-scan selection of the first num_pos positives / num_neg
negatives, and the two masked row sums.  The host does the constant
score-order gather and the final 128-row scalar math.
"""

import numpy as np

B, N, C = 128, 131072, 2
NCORES = 8
ROWS = B // NCORES  # 16 rows per core
K = 256             # score-order prefix depth per row

_cache = {}


def _perm():
    """[B, K] int64: first K positions of each row in score-descending order.

    Must match jax.lax.top_k tie-breaking on the reference's scores exactly,
    so compute it with jax.lax.top_k on the very same scores (CPU backend;
    threefry PRNG is backend-deterministic).
    """
    if "perm" not in _cache:
        import jax

        cpu = jax.devices("cpu")[0]
        with jax.default_device(cpu):
            scores = jax.random.uniform(jax.random.key(42), (B, N), dtype=jax.numpy.float32)
            _, idx = jax.lax.top_k(scores, K)
        _cache["perm"] = np.asarray(jax.device_get(idx)).astype(np.int64)
    return _cache["perm"]


def _build_nc(num_pos: int, num_neg: int):
    """Compile the single-core Bass program (same NEFF on all 8 cores)."""
    key = ("nc", num_pos, num_neg)
    if key in _cache:
        return _cache[key]

    import concourse.bacc as bacc
    import concourse.bass as bass
    import concourse.mybir as mybir
    import concourse.tile as tile

    dt = mybir.dt
    af = mybir.ActivationFunctionType
    alu = mybir.AluOpType

    # Steer the ACT-table pass: by default it picks `exp_and_others` for Exp
    # and `natural_log` for Ln, which evict each other (1.28us reload on the
    # critical path).  Restrict Exp/Ln to the combined
    # `natural_log_exp_and_others` set (keeping every set's index intact so
    # act_func_set_id stays valid) -> a single table load serves both.
    if not _cache.get("act_tables_patched"):
        orig_get = bacc.get_activation_tables

        def _combined_tables(arch):
            tabs = orig_get(arch)
            combined = "natural_log_exp_and_others"
            if combined in tabs and {af.Exp, af.Ln} <= tabs[combined]:
                for name, fns in tabs.items():
                    if name != combined:
                        fns.discard(af.Exp)
                        fns.discard(af.Ln)
            return tabs

        bacc.get_activation_tables = _combined_tables
        _cache["act_tables_patched"] = True

    nc = bacc.Bacc("TRN2", target_bir_lowering=False, debug=False)

    # pk packs [t | 2t-1 | 1-t | iota | x0 | x1] along the free dim
    # (the affine transforms of the gathered target are host-trivial and
    # save three DVE ops on the critical path)
    pk = nc.dram_tensor("pk", [ROWS, 6 * K], dt.float32, kind="ExternalInput")
    out = nc.dram_tensor("out", [ROWS, 2], dt.float32, kind="ExternalOutput")

    with tile.TileContext(nc) as tc:
        with tc.tile_pool(name="small", bufs=1) as sp:
            zeros = sp.tile([ROWS, K], dt.float32)
            nc.gpsimd.memset(zeros[:], 0.0)

            pkt = sp.tile([ROWS, 6 * K], dt.float32)
            nc.sync.dma_start(pkt[:], pk.ap())
            tf = pkt[:, 0:K]
            sg = pkt[:, K:2 * K]
            tn = pkt[:, 2 * K:3 * K]
            iota = pkt[:, 3 * K:4 * K]
            x0 = pkt[:, 4 * K:5 * K]
            x1 = pkt[:, 5 * K:6 * K]

            # ce chain (DVE feeds ACT as early as possible)
            d = sp.tile([ROWS, K], dt.float32)
            nc.vector.tensor_sub(d[:], x0, x1)
            dd = sp.tile([ROWS, K], dt.float32)
            nc.vector.tensor_mul(dd[:], d[:], sg)
            # softplus(dd) = relu(dd) + ln(1 + exp(-|dd|)); abs/exp/ln on ACT
            rl = sp.tile([ROWS, K], dt.float32)
            nc.vector.tensor_scalar_max(rl[:], dd[:], 0.0)
            ab = sp.tile([ROWS, K], dt.float32)
            nc.scalar.activation(ab[:], dd[:], af.Abs)
            ex = sp.tile([ROWS, K], dt.float32)
            nc.scalar.activation(ex[:], ab[:], af.Exp, scale=-1.0)
            ln = sp.tile([ROWS, K], dt.float32)
            nc.scalar.activation(ln[:], ex[:], af.Ln, bias=1.0)

            # selection masks (need only tf -- run on DVE while ACT works)
            cpos = sp.tile([ROWS, K], dt.float32)
            nc.vector.tensor_tensor_scan(
                cpos[:], tf, zeros[:], 0.0, op0=alu.add, op1=alu.add
            )
            cneg = sp.tile([ROWS, K], dt.float32)
            nc.vector.scalar_tensor_tensor(
                cneg[:], cpos[:], -1.0, iota, op0=alu.mult, op1=alu.add
            )
            maskp = sp.tile([ROWS, K], dt.float32)
            nc.vector.scalar_tensor_tensor(
                maskp[:], cpos[:], float(num_pos), tf, op0=alu.is_le, op1=alu.mult
            )
            maskn = sp.tile([ROWS, K], dt.float32)
            nc.vector.scalar_tensor_tensor(
                maskn[:], cneg[:], float(num_neg), tn, op0=alu.is_le, op1=alu.mult
            )

            ce = sp.tile([ROWS, K], dt.float32)
            nc.vector.tensor_add(ce[:], rl[:], ln[:])

            outsb = sp.tile([ROWS, 2], dt.float32)
            junk0 = sp.tile([ROWS, K], dt.float32)
            nc.vector.scalar_tensor_tensor(
                junk0[:], ce[:], 1.0, maskp[:],
                op0=alu.mult, op1=alu.mult, accum_out=outsb[:, 0:1],
            )
            junk1 = sp.tile([ROWS, K], dt.float32)
            nc.vector.scalar_tensor_tensor(
                junk1[:], ce[:], 1.0, maskn[:],
                op0=alu.mult, op1=alu.mult, accum_out=outsb[:, 1:2],
            )

            nc.sync.dma_start(out.ap(), outsb[:])

    nc.compile()
    _cache[key] = nc
    return nc


def _host_exact(inputs, target, num_pos, num_neg):
    """Exact replication of the reference (jax on CPU). Safety fallback only."""
    import jax
    import jax.numpy as jnp

    cpu = jax.devices("cpu")[0]
    with jax.default_device(cpu):
        inputs = jnp.asarray(inputs)
        target = jnp.asarray(target)
        scores = jax.random.uniform(jax.random.key(42), (B, N))
        is_pos = target == 1
        is_neg = target == 0
        count_pos = is_pos.sum(axis=-1)
        min_pos = jnp.minimum(count_pos, num_pos)
        min_neg = jnp.minimum((count_pos * num_neg) // num_pos, num_neg)
        logp = jax.nn.log_softmax(inputs, axis=-1)
        ce = -jnp.take_along_axis(logp, target[..., None], axis=-1)[..., 0]

        def sampled_mean(mask, k, min_k):
            s = jnp.where(mask, scores, -jnp.inf)
            _, idx = jax.lax.top_k(s, k)
            sel = jnp.take_along_axis(ce, idx, axis=-1)
            valid = jnp.arange(k)[None, :] < min_k[:, None]
            return jnp.where(valid, sel, 0.0).sum(axis=-1) / jnp.maximum(min_k, 1)

        pos_loss = sampled_mean(is_pos, num_pos, min_pos)
        neg_loss = sampled_mean(is_neg, num_neg, min_neg)
        res = ((pos_loss + neg_loss) * 0.5).mean()
    return np.asarray(jax.device_get(res)).astype(np.float32)


def kernel(**inputs) -> np.ndarray:
    from concourse.bass_utils import run_bass_kernel_spmd

    x = np.ascontiguousarray(np.asarray(inputs["inputs"], dtype=np.float32))
    target = np.ascontiguousarray(np.asarray(inputs["target"], dtype=np.int32))
    num_pos = int(np.asarray(inputs["num_pos"]))
    num_neg = int(np.asarray(inputs["num_neg"]))

    if num_pos <= 0 or num_pos > K or num_neg < 0 or num_neg > K:
        # degenerate configs the device program doesn't cover
        return _host_exact(x, target, num_pos, num_neg)

    perm = _perm()
    gt = np.take_along_axis(target, perm, axis=1)          # [B, K] int32
    # Guard: with >= num_pos positives and >= num_neg negatives inside every
    # row's K-prefix, min_pos == num_pos and min_neg == num_neg exactly
    # ((c*nn)//np >= nn  <=>  c >= np for nn > 0), the selected samples all
    # lie inside the prefix, and count_pos is never needed.  Fall back to
    # the exact host computation otherwise (never fires for this data:
    # binomial(256, 1/2) tails; real-data margins are >= 100 of each).
    prefix_pos = gt.sum(axis=1, dtype=np.int64)
    prefix_neg = K - prefix_pos
    if (prefix_pos < num_pos).any() or (prefix_neg < num_neg).any():
        return _host_exact(x, target, num_pos, num_neg)

    pk = np.empty((B, 6 * K), dtype=np.float32)
    pk[:, 0:K] = gt
    pk[:, K:2 * K] = 2.0 * pk[:, 0:K] - 1.0
    pk[:, 2 * K:3 * K] = 1.0 - pk[:, 0:K]
    pk[:, 3 * K:4 * K] = np.arange(1, K + 1, dtype=np.float32)
    pk[:, 4 * K:5 * K] = np.take_along_axis(x[:, :, 0], perm, axis=1)
    pk[:, 5 * K:6 * K] = np.take_along_axis(x[:, :, 1], perm, axis=1)

    nc = _build_nc(num_pos, num_neg)
    core_ids = list(range(NCORES))
    in_maps = [
        {"pk": np.ascontiguousarray(pk[c * ROWS:(c + 1) * ROWS])}
        for c in core_ids
    ]
    res = run_bass_kernel_spmd(nc, in_maps, core_ids, trace=_cache.get("trace", False))
    _cache["last_res"] = res
    outs = np.concatenate([res.results[c]["out"] for c in core_ids], axis=0)  # [B,2]

    pos_loss = outs[:, 0].astype(np.float32) / np.float32(num_pos)
    neg_loss = outs[:, 1].astype(np.float32) / np.float32(max(num_neg, 1))
    loss = np.float32(0.5) * (pos_loss + neg_loss)
    return np.asarray(loss.mean(), dtype=np.float32)


# revision 37
# speedup vs baseline: 1.2345x; 1.1426x over previous
"""Balanced CE loss kernel for Trainium2 (8 NeuronCores, data parallel).

Math recap of the reference:
  - ce[b,n] = -log_softmax(inputs[b,n,:2])[target[b,n]]
            = softplus((x0-x1) * (2*t-1))           (two-class CE)
  - scores = uniform(key(42), (B,N))  -- a COMPILE-TIME CONSTANT
  - per row: mean of ce over the top-`num_pos`-by-score positives and the
    top-`num_neg`-by-score negatives; valid-count capped by count_pos.
  - loss = mean_b 0.5 * (pos_mean + neg_mean)

Key reductions:
  1. Only positions among each row's top-K (K=256) constant score order can
     be selected, so only those positions of inputs/target matter.
  2. count_pos only enters via min(count_pos, num_pos) and
     min((count_pos*num_neg)//num_pos, num_neg).  If the K-prefix already
     holds >= num_pos positives and >= num_neg negatives (checked EXACTLY on
     the host from the gathered prefix; bit-exact fallback otherwise), both
     saturate to num_pos / num_neg and the full count is never needed.

So each core only computes, for its 16 rows: ce over the K-prefix, a
hardware prefix-scan selection of the first num_pos positives / num_neg
negatives, and the two masked row sums.  The host does the constant
score-order gather and the final 128-row scalar math.
"""

import numpy as np

B, N, C = 128, 131072, 2
NCORES = 8
ROWS = B // NCORES  # 16 rows per core
K = 256             # score-order prefix depth per row

_cache = {}


def _perm():
    """[B, K] int64: first K positions of each row in score-descending order.

    Must match jax.lax.top_k tie-breaking on the reference's scores exactly,
    so compute it with jax.lax.top_k on the very same scores (CPU backend;
    threefry PRNG is backend-deterministic).
    """
    if "perm" not in _cache:
        import jax

        cpu = jax.devices("cpu")[0]
        with jax.default_device(cpu):
            scores = jax.random.uniform(jax.random.key(42), (B, N), dtype=jax.numpy.float32)
            _, idx = jax.lax.top_k(scores, K)
        _cache["perm"] = np.asarray(jax.device_get(idx)).astype(np.int64)
    return _cache["perm"]


def _build_nc(num_pos: int, num_neg: int):
    """Compile the single-core Bass program (same NEFF on all 8 cores)."""
    key = ("nc", num_pos, num_neg)
    if key in _cache:
        return _cache[key]

    import concourse.bacc as bacc
    import concourse.bass as bass
    import concourse.mybir as mybir
    import concourse.tile as tile

    dt = mybir.dt
    af = mybir.ActivationFunctionType
    alu = mybir.AluOpType

    # Steer the ACT-table pass: by default it picks `exp_and_others` for Exp
    # and `natural_log` for Ln, which evict each other (1.28us reload on the
    # critical path).  Restrict Exp/Ln to the combined
    # `natural_log_exp_and_others` set (keeping every set's index intact so
    # act_func_set_id stays valid) -> a single table load serves both.
    if not _cache.get("act_tables_patched"):
        orig_get = bacc.get_activation_tables

        def _combined_tables(arch):
            tabs = orig_get(arch)
            combined = "natural_log_exp_and_others"
            if combined in tabs and {af.Exp, af.Ln} <= tabs[combined]:
                for name, fns in tabs.items():
                    if name != combined:
                        fns.discard(af.Exp)
                        fns.discard(af.Ln)
            return tabs

        bacc.get_activation_tables = _combined_tables
        _cache["act_tables_patched"] = True

    nc = bacc.Bacc("TRN2", target_bir_lowering=False, debug=False)

    # Two packed inputs on separate DMA queues (sync / gpsimd) so both
    # halves land ~in parallel: pk1 = [2t-1 | x0 | x1] feeds the ce chain,
    # pk2 = [t | 1-t | iota] feeds the selection chain.
    pk1 = nc.dram_tensor("pk1", [ROWS, 3 * K], dt.float32, kind="ExternalInput")
    pk2 = nc.dram_tensor("pk2", [ROWS, 3 * K], dt.float32, kind="ExternalInput")
    out = nc.dram_tensor("out", [ROWS, 2], dt.float32, kind="ExternalOutput")

    with tile.TileContext(nc) as tc:
        with tc.tile_pool(name="small", bufs=1) as sp:
            zeros = sp.tile([ROWS, K], dt.float32)
            nc.gpsimd.memset(zeros[:], 0.0)

            pkt1 = sp.tile([ROWS, 3 * K], dt.float32)
            nc.sync.dma_start(pkt1[:], pk1.ap())
            pkt2 = sp.tile([ROWS, 3 * K], dt.float32)
            nc.gpsimd.dma_start(pkt2[:], pk2.ap())
            sg = pkt1[:, 0:K]
            x0 = pkt1[:, K:2 * K]
            x1 = pkt1[:, 2 * K:3 * K]
            tf = pkt2[:, 0:K]
            tn = pkt2[:, K:2 * K]
            iota = pkt2[:, 2 * K:3 * K]

            # ce chain (DVE feeds ACT as early as possible)
            d = sp.tile([ROWS, K], dt.float32)
            nc.vector.tensor_sub(d[:], x0, x1)
            dd = sp.tile([ROWS, K], dt.float32)
            nc.vector.tensor_mul(dd[:], d[:], sg)
            # ce = softplus(dd) = ln(1 + exp(dd)) computed directly: the host
            # guards max|x0-x1| < 80 over the prefix (exact fallback
            # otherwise), so exp cannot overflow.
            ex = sp.tile([ROWS, K], dt.float32)
            nc.scalar.activation(ex[:], dd[:], af.Exp)
            ln = sp.tile([ROWS, K], dt.float32)
            nc.scalar.activation(ln[:], ex[:], af.Ln, bias=1.0)

            # selection masks (need only tf -- run on DVE while ACT works)
            cpos = sp.tile([ROWS, K], dt.float32)
            nc.vector.tensor_tensor_scan(
                cpos[:], tf, zeros[:], 0.0, op0=alu.add, op1=alu.add
            )
            cneg = sp.tile([ROWS, K], dt.float32)
            nc.vector.scalar_tensor_tensor(
                cneg[:], cpos[:], -1.0, iota, op0=alu.mult, op1=alu.add
            )
            maskp = sp.tile([ROWS, K], dt.float32)
            nc.vector.scalar_tensor_tensor(
                maskp[:], cpos[:], float(num_pos), tf, op0=alu.is_le, op1=alu.mult
            )
            maskn = sp.tile([ROWS, K], dt.float32)
            nc.vector.scalar_tensor_tensor(
                maskn[:], cneg[:], float(num_neg), tn, op0=alu.is_le, op1=alu.mult
            )

            ce = ln
            outsb = sp.tile([ROWS, 2], dt.float32)
            junk0 = sp.tile([ROWS, K], dt.float32)
            nc.vector.scalar_tensor_tensor(
                junk0[:], ce[:], 1.0, maskp[:],
                op0=alu.mult, op1=alu.mult, accum_out=outsb[:, 0:1],
            )
            junk1 = sp.tile([ROWS, K], dt.float32)
            nc.vector.scalar_tensor_tensor(
                junk1[:], ce[:], 1.0, maskn[:],
                op0=alu.mult, op1=alu.mult, accum_out=outsb[:, 1:2],
            )

            nc.sync.dma_start(out.ap(), outsb[:])

    nc.compile()
    _cache[key] = nc
    return nc


def _host_exact(inputs, target, num_pos, num_neg):
    """Exact replication of the reference (jax on CPU). Safety fallback only."""
    import jax
    import jax.numpy as jnp

    cpu = jax.devices("cpu")[0]
    with jax.default_device(cpu):
        inputs = jnp.asarray(inputs)
        target = jnp.asarray(target)
        scores = jax.random.uniform(jax.random.key(42), (B, N))
        is_pos = target == 1
        is_neg = target == 0
        count_pos = is_pos.sum(axis=-1)
        min_pos = jnp.minimum(count_pos, num_pos)
        min_neg = jnp.minimum((count_pos * num_neg) // num_pos, num_neg)
        logp = jax.nn.log_softmax(inputs, axis=-1)
        ce = -jnp.take_along_axis(logp, target[..., None], axis=-1)[..., 0]

        def sampled_mean(mask, k, min_k):
            s = jnp.where(mask, scores, -jnp.inf)
            _, idx = jax.lax.top_k(s, k)
            sel = jnp.take_along_axis(ce, idx, axis=-1)
            valid = jnp.arange(k)[None, :] < min_k[:, None]
            return jnp.where(valid, sel, 0.0).sum(axis=-1) / jnp.maximum(min_k, 1)

        pos_loss = sampled_mean(is_pos, num_pos, min_pos)
        neg_loss = sampled_mean(is_neg, num_neg, min_neg)
        res = ((pos_loss + neg_loss) * 0.5).mean()
    return np.asarray(jax.device_get(res)).astype(np.float32)


def kernel(**inputs) -> np.ndarray:
    from concourse.bass_utils import run_bass_kernel_spmd

    x = np.ascontiguousarray(np.asarray(inputs["inputs"], dtype=np.float32))
    target = np.ascontiguousarray(np.asarray(inputs["target"], dtype=np.int32))
    num_pos = int(np.asarray(inputs["num_pos"]))
    num_neg = int(np.asarray(inputs["num_neg"]))

    if num_pos <= 0 or num_pos > K or num_neg < 0 or num_neg > K:
        # degenerate configs the device program doesn't cover
        return _host_exact(x, target, num_pos, num_neg)

    perm = _perm()
    gt = np.take_along_axis(target, perm, axis=1)          # [B, K] int32
    # Guard: with >= num_pos positives and >= num_neg negatives inside every
    # row's K-prefix, min_pos == num_pos and min_neg == num_neg exactly
    # ((c*nn)//np >= nn  <=>  c >= np for nn > 0), the selected samples all
    # lie inside the prefix, and count_pos is never needed.  Fall back to
    # the exact host computation otherwise (never fires for this data:
    # binomial(256, 1/2) tails; real-data margins are >= 100 of each).
    prefix_pos = gt.sum(axis=1, dtype=np.int64)
    prefix_neg = K - prefix_pos
    if (prefix_pos < num_pos).any() or (prefix_neg < num_neg).any():
        return _host_exact(x, target, num_pos, num_neg)

    gx0 = np.take_along_axis(x[:, :, 0], perm, axis=1)
    gx1 = np.take_along_axis(x[:, :, 1], perm, axis=1)
    if not np.isfinite(gx0).all() or not np.isfinite(gx1).all() or \
            np.abs(gx0 - gx1).max() >= 80.0:
        # exp(dd) on device must not overflow; never fires for randn inputs
        return _host_exact(x, target, num_pos, num_neg)
    gtf = gt.astype(np.float32)
    pk1 = np.empty((B, 3 * K), dtype=np.float32)
    pk1[:, 0:K] = 2.0 * gtf - 1.0
    pk1[:, K:2 * K] = gx0
    pk1[:, 2 * K:3 * K] = gx1
    pk2 = np.empty((B, 3 * K), dtype=np.float32)
    pk2[:, 0:K] = gtf
    pk2[:, K:2 * K] = 1.0 - gtf
    pk2[:, 2 * K:3 * K] = np.arange(1, K + 1, dtype=np.float32)

    nc = _build_nc(num_pos, num_neg)
    core_ids = list(range(NCORES))
    in_maps = [
        {
            "pk1": np.ascontiguousarray(pk1[c * ROWS:(c + 1) * ROWS]),
            "pk2": np.ascontiguousarray(pk2[c * ROWS:(c + 1) * ROWS]),
        }
        for c in core_ids
    ]
    res = run_bass_kernel_spmd(nc, in_maps, core_ids, trace=_cache.get("trace", False))
    _cache["last_res"] = res
    outs = np.concatenate([res.results[c]["out"] for c in core_ids], axis=0)  # [B,2]

    pos_loss = outs[:, 0].astype(np.float32) / np.float32(num_pos)
    neg_loss = outs[:, 1].astype(np.float32) / np.float32(max(num_neg, 1))
    loss = np.float32(0.5) * (pos_loss + neg_loss)
    return np.asarray(loss.mean(), dtype=np.float32)


# revision 38
# speedup vs baseline: 1.3030x; 1.0555x over previous
"""Balanced CE loss kernel for Trainium2 (8 NeuronCores, data parallel).

Math recap of the reference:
  - ce[b,n] = -log_softmax(inputs[b,n,:2])[target[b,n]]
            = softplus((x0-x1) * (2*t-1))           (two-class CE)
  - scores = uniform(key(42), (B,N))  -- a COMPILE-TIME CONSTANT
  - per row: mean of ce over the top-`num_pos`-by-score positives and the
    top-`num_neg`-by-score negatives; valid-count capped by count_pos.
  - loss = mean_b 0.5 * (pos_mean + neg_mean)

Key reductions:
  1. Only positions among each row's top-K (K=256) constant score order can
     be selected, so only those positions of inputs/target matter.
  2. count_pos only enters via min(count_pos, num_pos) and
     min((count_pos*num_neg)//num_pos, num_neg).  If the K-prefix already
     holds >= num_pos positives and >= num_neg negatives (checked EXACTLY on
     the host from the gathered prefix; bit-exact fallback otherwise), both
     saturate to num_pos / num_neg and the full count is never needed.

So each core only computes, for its 16 rows: ce over the K-prefix, a
hardware prefix-scan selection of the first num_pos positives / num_neg
negatives, and the two masked row sums.  The host does the constant
score-order gather and the final 128-row scalar math.
"""

import numpy as np

B, N, C = 128, 131072, 2
NCORES = 8
ROWS = B // NCORES  # 16 rows per core
K = 192             # score-order prefix depth per row

_cache = {}


def _perm():
    """[B, K] int64: first K positions of each row in score-descending order.

    Must match jax.lax.top_k tie-breaking on the reference's scores exactly,
    so compute it with jax.lax.top_k on the very same scores (CPU backend;
    threefry PRNG is backend-deterministic).
    """
    if "perm" not in _cache:
        import jax

        cpu = jax.devices("cpu")[0]
        with jax.default_device(cpu):
            scores = jax.random.uniform(jax.random.key(42), (B, N), dtype=jax.numpy.float32)
            _, idx = jax.lax.top_k(scores, K)
        _cache["perm"] = np.asarray(jax.device_get(idx)).astype(np.int64)
    return _cache["perm"]


def _build_nc(num_pos: int, num_neg: int):
    """Compile the single-core Bass program (same NEFF on all 8 cores)."""
    key = ("nc", num_pos, num_neg)
    if key in _cache:
        return _cache[key]

    import concourse.bacc as bacc
    import concourse.bass as bass
    import concourse.mybir as mybir
    import concourse.tile as tile

    dt = mybir.dt
    af = mybir.ActivationFunctionType
    alu = mybir.AluOpType

    # Steer the ACT-table pass: by default it picks `exp_and_others` for Exp
    # and `natural_log` for Ln, which evict each other (1.28us reload on the
    # critical path).  Restrict Exp/Ln to the combined
    # `natural_log_exp_and_others` set (keeping every set's index intact so
    # act_func_set_id stays valid) -> a single table load serves both.
    if not _cache.get("act_tables_patched"):
        orig_get = bacc.get_activation_tables

        def _combined_tables(arch):
            tabs = orig_get(arch)
            combined = "natural_log_exp_and_others"
            if combined in tabs and {af.Exp, af.Ln} <= tabs[combined]:
                for name, fns in tabs.items():
                    if name != combined:
                        fns.discard(af.Exp)
                        fns.discard(af.Ln)
            return tabs

        bacc.get_activation_tables = _combined_tables
        _cache["act_tables_patched"] = True

    nc = bacc.Bacc("TRN2", target_bir_lowering=False, debug=False)

    # Two packed inputs on separate DMA queues (sync / gpsimd) so both
    # halves land ~in parallel: pk1 = [x_other | x_target] (per-element class
    # gather done host-side -- pure indexing) feeds the ce chain,
    # pk2 = [t | 1-t | iota] feeds the selection chain.
    pk1 = nc.dram_tensor("pk1", [ROWS, 2 * K], dt.float32, kind="ExternalInput")
    pk2 = nc.dram_tensor("pk2", [ROWS, 3 * K], dt.float32, kind="ExternalInput")
    out = nc.dram_tensor("out", [ROWS, 2], dt.float32, kind="ExternalOutput")

    with tile.TileContext(nc) as tc:
        with tc.tile_pool(name="small", bufs=1) as sp:
            zeros = sp.tile([ROWS, K], dt.float32)
            nc.gpsimd.memset(zeros[:], 0.0)

            pkt1 = sp.tile([ROWS, 2 * K], dt.float32)
            nc.sync.dma_start(pkt1[:], pk1.ap())
            pkt2 = sp.tile([ROWS, 3 * K], dt.float32)
            nc.gpsimd.dma_start(pkt2[:], pk2.ap())
            xo = pkt1[:, 0:K]
            xt = pkt1[:, K:2 * K]
            tf = pkt2[:, 0:K]
            tn = pkt2[:, K:2 * K]
            iota = pkt2[:, 2 * K:3 * K]

            # ce chain (DVE feeds ACT as early as possible)
            dd = sp.tile([ROWS, K], dt.float32)
            nc.vector.tensor_sub(dd[:], xo, xt)
            # ce = softplus(dd) = ln(1 + exp(dd)) computed directly: the host
            # guards max|x0-x1| < 80 over the prefix (exact fallback
            # otherwise), so exp cannot overflow.
            ex = sp.tile([ROWS, K], dt.float32)
            nc.scalar.activation(ex[:], dd[:], af.Exp)
            ln = sp.tile([ROWS, K], dt.float32)
            nc.scalar.activation(ln[:], ex[:], af.Ln, bias=1.0)

            # selection masks (need only tf -- run on DVE while ACT works)
            cpos = sp.tile([ROWS, K], dt.float32)
            nc.vector.tensor_tensor_scan(
                cpos[:], tf, zeros[:], 0.0, op0=alu.add, op1=alu.add
            )
            cneg = sp.tile([ROWS, K], dt.float32)
            nc.vector.scalar_tensor_tensor(
                cneg[:], cpos[:], -1.0, iota, op0=alu.mult, op1=alu.add
            )
            maskp = sp.tile([ROWS, K], dt.float32)
            nc.vector.scalar_tensor_tensor(
                maskp[:], cpos[:], float(num_pos), tf, op0=alu.is_le, op1=alu.mult
            )
            maskn = sp.tile([ROWS, K], dt.float32)
            nc.vector.scalar_tensor_tensor(
                maskn[:], cneg[:], float(num_neg), tn, op0=alu.is_le, op1=alu.mult
            )

            ce = ln
            outsb = sp.tile([ROWS, 2], dt.float32)
            junk0 = sp.tile([ROWS, K], dt.float32)
            nc.vector.scalar_tensor_tensor(
                junk0[:], ce[:], 1.0, maskp[:],
                op0=alu.mult, op1=alu.mult, accum_out=outsb[:, 0:1],
            )
            junk1 = sp.tile([ROWS, K], dt.float32)
            nc.vector.scalar_tensor_tensor(
                junk1[:], ce[:], 1.0, maskn[:],
                op0=alu.mult, op1=alu.mult, accum_out=outsb[:, 1:2],
            )

            nc.sync.dma_start(out.ap(), outsb[:])

    nc.compile()
    _cache[key] = nc
    return nc


def _host_exact(inputs, target, num_pos, num_neg):
    """Exact replication of the reference (jax on CPU). Safety fallback only."""
    import jax
    import jax.numpy as jnp

    cpu = jax.devices("cpu")[0]
    with jax.default_device(cpu):
        inputs = jnp.asarray(inputs)
        target = jnp.asarray(target)
        scores = jax.random.uniform(jax.random.key(42), (B, N))
        is_pos = target == 1
        is_neg = target == 0
        count_pos = is_pos.sum(axis=-1)
        min_pos = jnp.minimum(count_pos, num_pos)
        min_neg = jnp.minimum((count_pos * num_neg) // num_pos, num_neg)
        logp = jax.nn.log_softmax(inputs, axis=-1)
        ce = -jnp.take_along_axis(logp, target[..., None], axis=-1)[..., 0]

        def sampled_mean(mask, k, min_k):
            s = jnp.where(mask, scores, -jnp.inf)
            _, idx = jax.lax.top_k(s, k)
            sel = jnp.take_along_axis(ce, idx, axis=-1)
            valid = jnp.arange(k)[None, :] < min_k[:, None]
            return jnp.where(valid, sel, 0.0).sum(axis=-1) / jnp.maximum(min_k, 1)

        pos_loss = sampled_mean(is_pos, num_pos, min_pos)
        neg_loss = sampled_mean(is_neg, num_neg, min_neg)
        res = ((pos_loss + neg_loss) * 0.5).mean()
    return np.asarray(jax.device_get(res)).astype(np.float32)


def kernel(**inputs) -> np.ndarray:
    from concourse.bass_utils import run_bass_kernel_spmd

    x = np.ascontiguousarray(np.asarray(inputs["inputs"], dtype=np.float32))
    target = np.ascontiguousarray(np.asarray(inputs["target"], dtype=np.int32))
    num_pos = int(np.asarray(inputs["num_pos"]))
    num_neg = int(np.asarray(inputs["num_neg"]))

    if num_pos <= 0 or num_pos > K or num_neg < 0 or num_neg > K:
        # degenerate configs the device program doesn't cover
        return _host_exact(x, target, num_pos, num_neg)

    perm = _perm()
    gt = np.take_along_axis(target, perm, axis=1)          # [B, K] int32
    # Guard: with >= num_pos positives and >= num_neg negatives inside every
    # row's K-prefix, min_pos == num_pos and min_neg == num_neg exactly
    # ((c*nn)//np >= nn  <=>  c >= np for nn > 0), the selected samples all
    # lie inside the prefix, and count_pos is never needed.  Fall back to
    # the exact host computation otherwise (never fires for this data:
    # binomial(256, 1/2) tails; real-data margins are >= 100 of each).
    prefix_pos = gt.sum(axis=1, dtype=np.int64)
    prefix_neg = K - prefix_pos
    if (prefix_pos < num_pos).any() or (prefix_neg < num_neg).any():
        return _host_exact(x, target, num_pos, num_neg)

    gx0 = np.take_along_axis(x[:, :, 0], perm, axis=1)
    gx1 = np.take_along_axis(x[:, :, 1], perm, axis=1)
    if not np.isfinite(gx0).all() or not np.isfinite(gx1).all() or \
            np.abs(gx0 - gx1).max() >= 80.0:
        # exp(dd) on device must not overflow; never fires for randn inputs
        return _host_exact(x, target, num_pos, num_neg)
    gtf = gt.astype(np.float32)
    isp = gt == 1
    pk1 = np.empty((B, 2 * K), dtype=np.float32)
    pk1[:, 0:K] = np.where(isp, gx0, gx1)     # x_other
    pk1[:, K:2 * K] = np.where(isp, gx1, gx0)  # x_target
    pk2 = np.empty((B, 3 * K), dtype=np.float32)
    pk2[:, 0:K] = gtf
    pk2[:, K:2 * K] = 1.0 - gtf
    pk2[:, 2 * K:3 * K] = np.arange(1, K + 1, dtype=np.float32)

    nc = _build_nc(num_pos, num_neg)
    core_ids = list(range(NCORES))
    in_maps = [
        {
            "pk1": np.ascontiguousarray(pk1[c * ROWS:(c + 1) * ROWS]),
            "pk2": np.ascontiguousarray(pk2[c * ROWS:(c + 1) * ROWS]),
        }
        for c in core_ids
    ]
    res = run_bass_kernel_spmd(nc, in_maps, core_ids, trace=_cache.get("trace", False))
    _cache["last_res"] = res
    outs = np.concatenate([res.results[c]["out"] for c in core_ids], axis=0)  # [B,2]

    pos_loss = outs[:, 0].astype(np.float32) / np.float32(num_pos)
    neg_loss = outs[:, 1].astype(np.float32) / np.float32(max(num_neg, 1))
    loss = np.float32(0.5) * (pos_loss + neg_loss)
    return np.asarray(loss.mean(), dtype=np.float32)
